# revision 9
# baseline (speedup 1.0000x reference)
"""FFTPatchEmbed Trainium2 kernel.

Computes: per-16x16-patch 2D rFFT (forward norm) -> log|.| -> linear
projection (432->768) -> abs, for x [32,3,512,512]. Data-parallel over
batch across 8 NeuronCores.

Math folding:
  f_ref = ln|Y/256| = 0.5*ln(Yr^2+Yi^2) - ln(256)   (Y computed unnormalized)
  out   = |f_ref @ W.T + b|
        = |ln(s) @ (0.5*W).T + (b - ln(256)*W.sum(1))|
so the device only computes s = Yr^2 + Yi^2, ln(s), one matmul, abs.

The 2D rFFT of a 16x16 patch is a single linear map G [288, 256]
(144 complex outputs as 144 real + 144 imag rows). Host reorganizes x to
patch-pixel-major [256, 1024] per image so all device DMAs are contiguous.
"""

import numpy as np

import concourse.bass as bass
import concourse.bacc as bacc
import concourse.mybir as mybir
import concourse.tile as tile
from concourse.bass_utils import run_bass_kernel_spmd
from contextlib import ExitStack

F32 = mybir.dt.float32
F32R = mybir.dt.float32r
AF = mybir.ActivationFunctionType
ALU = mybir.AluOpType

N_CORES = 8
BL = 4          # batches per core
C = 3
E = 768
NPAT = 1024     # patches per image (32x32)
NF = 144        # freq bins per patch per channel (16 x 9)

_cached = {}


def _build_g_matrices():
    jj = np.arange(16)
    Du = np.exp(-2j * np.pi * np.outer(np.arange(16), jj) / 16)  # [u, j]
    Rv = np.exp(-2j * np.pi * np.outer(np.arange(9), jj) / 16)   # [v, k]
    Gc = np.einsum("uj,vk->uvjk", Du, Rv).reshape(144, 256)      # f=u*9+v, p=j*16+k
    Gr = Gc.real
    Gi = Gc.imag
    # lhsT column order: [Yr f0:128 | Yi f0:128 | Yr f128:144 , Yi f128:144]
    Gpack = np.concatenate([Gr[0:128], Gi[0:128], Gr[128:144], Gi[128:144]], 0)
    return np.ascontiguousarray(Gpack.T, dtype=np.float32)       # [256, 288]


def _build_program():
    if "nc" in _cached:
        return _cached["nc"]

    nc = bacc.Bacc(
        "TRN2",
        target_bir_lowering=False,
        debug=False,
        num_devices=N_CORES,
    )
    xg = nc.dram_tensor("xg", [BL, C, 256, NPAT], F32, kind="ExternalInput").ap()
    gt = nc.dram_tensor("gt", [256, 288], F32, kind="ExternalInput").ap()
    wt = nc.dram_tensor("wt", [432, E], F32R, kind="ExternalInput").ap()
    b2 = nc.dram_tensor("b2", [128, 6], F32, kind="ExternalInput").ap()
    sel = nc.dram_tensor("sel", [96, 48], F32R, kind="ExternalInput").ap()
    out = nc.dram_tensor("out", [BL, E, NPAT], F32, kind="ExternalOutput").ap()

    with tile.TileContext(nc) as tc, ExitStack() as ctx:
        wpool = ctx.enter_context(tc.tile_pool(name="weights", bufs=1))
        xpool = ctx.enter_context(tc.tile_pool(name="xin", bufs=1))
        sqpool = ctx.enter_context(tc.tile_pool(name="sq", bufs=3))
        spool = ctx.enter_context(tc.tile_pool(name="s", bufs=2))
        ppool = ctx.enter_context(tc.tile_pool(name="projrhs", bufs=2))
        opool = ctx.enter_context(tc.tile_pool(name="outsb", bufs=3))
        mainps = ctx.enter_context(tc.tile_pool(name="mainps", bufs=2, space="PSUM"))
        tailps = ctx.enter_context(tc.tile_pool(name="tailps", bufs=2, space="PSUM"))
        projps = ctx.enter_context(tc.tile_pool(name="projps", bufs=2, space="PSUM"))

        # Stationary operands, loaded once.
        g0 = wpool.tile([128, 288], F32, name="g0")
        g1 = wpool.tile([128, 288], F32, name="g1")
        nc.sync.dma_start(out=g0[:], in_=gt[0:128, :])
        nc.sync.dma_start(out=g1[:], in_=gt[128:256, :])
        ksz = [128, 128, 128, 48]
        kof = [0, 128, 256, 384]
        wk = []
        for i in range(4):
            t = wpool.tile([ksz[i], E], F32R, name=f"wk{i}")
            nc.sync.dma_start(out=t[:], in_=wt[kof[i] : kof[i] + ksz[i], :])
            wk.append(t)
        b2t = wpool.tile([128, 6], F32, name="b2t")
        nc.sync.dma_start(out=b2t[:], in_=b2[:, :])
        selt = wpool.tile([96, 48], F32R, name="selt")
        nc.sync.dma_start(out=selt[:], in_=sel[:, :])

        def mm(po, lhsT, rhs, start, stop):
            nc.tensor.matmul(po, lhsT, rhs, start=start, stop=stop)

        # Preload every input tile once: fresh DMA destinations carry no
        # slot-recycling waits (walrus caps sync waits per DMACopy), and all
        # input traffic streams ahead of compute.
        xt = {}
        for bb in range(BL):
            for cc in range(C):
                for q in range(2):
                    for nb in range(2):
                        t = xpool.tile(
                            [128, 512], F32, name=f"x{bb}{cc}{q}{nb}"
                        )
                        nc.sync.dma_start(
                            out=t[:],
                            in_=xg[
                                bb,
                                cc,
                                q * 128 : (q + 1) * 128,
                                nb * 512 : (nb + 1) * 512,
                            ],
                        )
                        xt[bb, cc, q, nb] = t

        for bb in range(BL):
            for nb in range(2):  # 512-patch column batches
                n0 = nb * 512
                pch = []
                for i in range(4):
                    pch.append(ppool.tile([ksz[i], 512], F32R, name=f"p{i}", tag=f"p{i}"))
                tails = [
                    tailps.tile([32, 512], F32, name=f"tail{i}", tag=f"tail{i}", bufs=1)
                    for i in range(C)
                ]
                for cc in range(C):
                    x0 = xt[bb, cc, 0, nb]
                    x1 = xt[bb, cc, 1, nb]
                    # tail freqs f128:144, Yr rows 0:16 / Yi rows 16:32
                    mm(tails[cc][:], g0[:, 256:288], x0[:], True, False)
                    mm(tails[cc][:], g1[:, 256:288], x1[:], False, True)
                    s_c = spool.tile([128, 512], F32, name="s_c", tag="s_c")
                    for h in range(2):  # 256-patch halves; Yr|Yi packed in free dim
                        ps = mainps.tile([128, 512], F32, name="mainp", tag="mainp")
                        xs0 = x0[:, h * 256 : (h + 1) * 256]
                        xs1 = x1[:, h * 256 : (h + 1) * 256]
                        mm(ps[:, 0:256], g0[:, 0:128], xs0, True, False)
                        mm(ps[:, 0:256], g1[:, 0:128], xs1, False, True)
                        mm(ps[:, 256:512], g0[:, 128:256], xs0, True, False)
                        mm(ps[:, 256:512], g1[:, 128:256], xs1, False, True)
                        sq = sqpool.tile([128, 512], F32, name="sqm", tag="sqm")
                        nc.scalar.activation(sq[:], ps[:], AF.Square)
                        nc.vector.tensor_add(
                            s_c[:, h * 256 : (h + 1) * 256],
                            sq[:, 0:256],
                            sq[:, 256:512],
                        )
                    nc.scalar.activation(pch[cc][:], s_c[:], AF.Ln)
                # tails: all 3 channels in one [96, 512] psum tile.
                # Pair-sum Yr^2+Yi^2 via a 0/1 selector matmul (PE) since
                # DVE cannot address 16-row partition offsets.
                sqt = sqpool.tile([96, 512], F32R, name="sqt", tag="sqt")
                for cc in range(C):
                    nc.scalar.activation(
                        sqt[32 * cc : 32 * cc + 32, :], tails[cc][:], AF.Square
                    )
                stp = tailps.tile([48, 512], F32, name="stp", tag="stp", bufs=1)
                mm(stp[:], selt[:], sqt[:], True, True)
                nc.scalar.activation(pch[3][:], stp[:], AF.Ln)
                # projection 432 -> 768, then |. + bias|
                for ec in range(6):
                    po = projps.tile([128, 512], F32, name="projp", tag="projp")
                    for kc in range(4):
                        mm(
                            po[:],
                            wk[kc][:, ec * 128 : (ec + 1) * 128],
                            pch[kc][:],
                            kc == 0,
                            kc == 3,
                        )
                    ob = opool.tile([128, 512], F32, name="ob", tag="ob")
                    nc.scalar.activation(
                        ob[:], po[:], AF.Abs, bias=b2t[:, ec : ec + 1]
                    )
                    nc.sync.dma_start(
                        out=out[bb, ec * 128 : (ec + 1) * 128, n0 : n0 + 512],
                        in_=ob[:],
                    )

    nc.compile()
    _cached["nc"] = nc
    return nc


def kernel(x, W, b):
    x = np.ascontiguousarray(np.asarray(x), dtype=np.float32)
    W = np.asarray(W, dtype=np.float32)
    b = np.asarray(b, dtype=np.float32)

    GT = _build_g_matrices()
    Wh = 0.5 * W  # [768, 432]
    perm = np.r_[0:128, 144:272, 288:416, 128:144, 272:288, 416:432]
    WT = np.ascontiguousarray(Wh.T[perm], dtype=np.float32)  # [432, 768]
    b2v = b - np.log(256.0) * W.sum(1)
    B2 = np.ascontiguousarray(b2v.reshape(6, 128).T, dtype=np.float32)  # [128, 6]
    SEL = np.zeros((96, 48), dtype=np.float32)
    for c in range(3):
        for r in range(16):
            SEL[32 * c + r, 16 * c + r] = 1.0
            SEL[32 * c + 16 + r, 16 * c + r] = 1.0

    nc = _build_program()
    in_maps = []
    for m in range(N_CORES):
        xs = x[m * BL : (m + 1) * BL]  # [4, 3, 512, 512]
        xgm = (
            xs.reshape(BL, C, 32, 16, 32, 16)
            .transpose(0, 1, 3, 5, 2, 4)
            .reshape(BL, C, 256, NPAT)
        )
        in_maps.append(
            {
                "xg": np.ascontiguousarray(xgm),
                "gt": GT,
                "wt": WT,
                "b2": B2,
                "sel": SEL,
            }
        )

    import os

    res = run_bass_kernel_spmd(
        nc,
        in_maps,
        list(range(N_CORES)),
        trace=bool(int(os.environ.get("KERNEL_TRACE", "0"))),
    )
    global LAST_RESULT
    LAST_RESULT = res
    outs = [res.results[m]["out"].reshape(BL, E, 32, 32) for m in range(N_CORES)]
    return np.concatenate(outs, 0)


LAST_RESULT = None


# revision 11
# speedup vs baseline: 1.0096x; 1.0096x over previous
"""FFTPatchEmbed Trainium2 kernel.

Computes: per-16x16-patch 2D rFFT (forward norm) -> log|.| -> linear
projection (432->768) -> abs, for x [32,3,512,512]. Data-parallel over
batch across 8 NeuronCores.

Math folding:
  f_ref = ln|Y/256| = 0.5*ln(Yr^2+Yi^2) - ln(256)   (Y computed unnormalized)
  out   = |f_ref @ W.T + b|
        = |ln(s) @ (0.5*W).T + (b - ln(256)*W.sum(1))|
so the device only computes s = Yr^2 + Yi^2, ln(s), one matmul, abs.

The 2D rFFT of a 16x16 patch is a single linear map G [288, 256]
(144 complex outputs as 144 real + 144 imag rows). Host reorganizes x to
patch-pixel-major [256, 1024] per image so all device DMAs are contiguous.
"""

import numpy as np

import concourse.bass as bass
import concourse.bacc as bacc
import concourse.mybir as mybir
import concourse.tile as tile
from concourse.bass_utils import run_bass_kernel_spmd
from contextlib import ExitStack

F32 = mybir.dt.float32
F32R = mybir.dt.float32r
AF = mybir.ActivationFunctionType
ALU = mybir.AluOpType

N_CORES = 8
BL = 4          # batches per core
C = 3
E = 768
NPAT = 1024     # patches per image (32x32)
NF = 144        # freq bins per patch per channel (16 x 9)

_cached = {}


def _build_g_matrices():
    jj = np.arange(16)
    Du = np.exp(-2j * np.pi * np.outer(np.arange(16), jj) / 16)  # [u, j]
    Rv = np.exp(-2j * np.pi * np.outer(np.arange(9), jj) / 16)   # [v, k]
    Gc = np.einsum("uj,vk->uvjk", Du, Rv).reshape(144, 256)      # f=u*9+v, p=j*16+k
    Gr = Gc.real
    Gi = Gc.imag
    # lhsT column order: [Yr f0:128 | Yi f0:128 | Yr f128:144 , Yi f128:144]
    Gpack = np.concatenate([Gr[0:128], Gi[0:128], Gr[128:144], Gi[128:144]], 0)
    return np.ascontiguousarray(Gpack.T, dtype=np.float32)       # [256, 288]


def _build_program():
    if "nc" in _cached:
        return _cached["nc"]

    nc = bacc.Bacc(
        "TRN2",
        target_bir_lowering=False,
        debug=False,
        num_devices=N_CORES,
    )
    xg = nc.dram_tensor("xg", [BL, C, 256, NPAT], F32, kind="ExternalInput").ap()
    gt = nc.dram_tensor("gt", [256, 288], F32, kind="ExternalInput").ap()
    wt = nc.dram_tensor("wt", [432, E], F32R, kind="ExternalInput").ap()
    b2 = nc.dram_tensor("b2", [128, 6], F32, kind="ExternalInput").ap()
    sel = nc.dram_tensor("sel", [96, 48], F32R, kind="ExternalInput").ap()
    out = nc.dram_tensor("out", [BL, E, NPAT], F32, kind="ExternalOutput").ap()

    with tile.TileContext(nc) as tc, ExitStack() as ctx:
        wpool = ctx.enter_context(tc.tile_pool(name="weights", bufs=1))
        xpool = ctx.enter_context(tc.tile_pool(name="xin", bufs=1))
        sqpool = ctx.enter_context(tc.tile_pool(name="sq", bufs=3))
        spool = ctx.enter_context(tc.tile_pool(name="s", bufs=2))
        ppool = ctx.enter_context(tc.tile_pool(name="projrhs", bufs=2))
        opool = ctx.enter_context(tc.tile_pool(name="outsb", bufs=3))
        mainps = ctx.enter_context(tc.tile_pool(name="mainps", bufs=2, space="PSUM"))
        tailps = ctx.enter_context(tc.tile_pool(name="tailps", bufs=2, space="PSUM"))
        projps = ctx.enter_context(tc.tile_pool(name="projps", bufs=2, space="PSUM"))

        # Stationary operands, loaded once.
        g0 = wpool.tile([128, 288], F32, name="g0")
        g1 = wpool.tile([128, 288], F32, name="g1")
        nc.sync.dma_start(out=g0[:], in_=gt[0:128, :])
        nc.sync.dma_start(out=g1[:], in_=gt[128:256, :])
        ksz = [128, 128, 128, 48]
        kof = [0, 128, 256, 384]
        wk = []
        for i in range(4):
            t = wpool.tile([ksz[i], E], F32R, name=f"wk{i}")
            nc.sync.dma_start(out=t[:], in_=wt[kof[i] : kof[i] + ksz[i], :])
            wk.append(t)
        b2t = wpool.tile([128, 6], F32, name="b2t")
        nc.sync.dma_start(out=b2t[:], in_=b2[:, :])
        selt = wpool.tile([96, 48], F32R, name="selt")
        nc.sync.dma_start(out=selt[:], in_=sel[:, :])

        def mm(po, lhsT, rhs, start, stop):
            nc.tensor.matmul(po, lhsT, rhs, start=start, stop=stop)

        # Preload every input tile once: fresh DMA destinations carry no
        # slot-recycling waits (walrus caps sync waits per DMACopy), and all
        # input traffic streams ahead of compute.
        xt = {}
        for bb in range(BL):
            for cc in range(C):
                for q in range(2):
                    for nb in range(2):
                        t = xpool.tile(
                            [128, 512], F32, name=f"x{bb}{cc}{q}{nb}"
                        )
                        nc.sync.dma_start(
                            out=t[:],
                            in_=xg[
                                bb,
                                cc,
                                q * 128 : (q + 1) * 128,
                                nb * 512 : (nb + 1) * 512,
                            ],
                        )
                        xt[bb, cc, q, nb] = t

        for bb in range(BL):
            for nb in range(2):  # 512-patch column batches
                n0 = nb * 512
                pch = []
                for i in range(4):
                    pch.append(ppool.tile([ksz[i], 512], F32R, name=f"p{i}", tag=f"p{i}"))
                tails = [
                    tailps.tile([32, 512], F32, name=f"tail{i}", tag=f"tail{i}", bufs=1)
                    for i in range(C)
                ]
                for cc in range(C):
                    x0 = xt[bb, cc, 0, nb]
                    x1 = xt[bb, cc, 1, nb]
                    # tail freqs f128:144, Yr rows 0:16 / Yi rows 16:32
                    mm(tails[cc][:], g0[:, 256:288], x0[:], True, False)
                    mm(tails[cc][:], g1[:, 256:288], x1[:], False, True)
                    s_c = spool.tile([128, 512], F32, name="s_c", tag="s_c")
                    for h in range(2):  # 256-patch halves; Yr|Yi packed in free dim
                        ps = mainps.tile([128, 512], F32, name="mainp", tag="mainp")
                        xs0 = x0[:, h * 256 : (h + 1) * 256]
                        xs1 = x1[:, h * 256 : (h + 1) * 256]
                        mm(ps[:, 0:256], g0[:, 0:128], xs0, True, False)
                        mm(ps[:, 0:256], g1[:, 0:128], xs1, False, True)
                        mm(ps[:, 256:512], g0[:, 128:256], xs0, True, False)
                        mm(ps[:, 256:512], g1[:, 128:256], xs1, False, True)
                        sq = sqpool.tile([128, 512], F32, name="sqm", tag="sqm")
                        nc.scalar.activation(sq[:], ps[:], AF.Square)
                        nc.vector.tensor_add(
                            s_c[:, h * 256 : (h + 1) * 256],
                            sq[:, 0:256],
                            sq[:, 256:512],
                        )
                    nc.scalar.activation(pch[cc][:], s_c[:], AF.Ln)
                # tails: all 3 channels in one [96, 512] psum tile.
                # Pair-sum Yr^2+Yi^2 via a 0/1 selector matmul (PE) since
                # DVE cannot address 16-row partition offsets.
                sqt = sqpool.tile([96, 512], F32R, name="sqt", tag="sqt")
                for cc in range(C):
                    nc.scalar.activation(
                        sqt[32 * cc : 32 * cc + 32, :], tails[cc][:], AF.Square
                    )
                stp = tailps.tile([48, 512], F32, name="stp", tag="stp", bufs=1)
                mm(stp[:], selt[:], sqt[:], True, True)
                nc.scalar.activation(pch[3][:], stp[:], AF.Ln)
                # projection 432 -> 768, then |. + bias|
                for ec in range(6):
                    po = projps.tile([128, 512], F32, name="projp", tag="projp")
                    for kc in range(4):
                        mm(
                            po[:],
                            wk[kc][:, ec * 128 : (ec + 1) * 128],
                            pch[kc][:],
                            kc == 0,
                            kc == 3,
                        )
                    ob = opool.tile([128, 512], F32, name="ob", tag="ob")
                    nc.scalar.activation(
                        ob[:], po[:], AF.Abs, bias=b2t[:, ec : ec + 1]
                    )
                    nc.sync.dma_start(
                        out=out[bb, ec * 128 : (ec + 1) * 128, n0 : n0 + 512],
                        in_=ob[:],
                    )

    nc.compile()
    _cached["nc"] = nc
    return nc


def kernel(x, W, b):
    x = np.ascontiguousarray(np.asarray(x), dtype=np.float32)
    W = np.asarray(W, dtype=np.float32)
    b = np.asarray(b, dtype=np.float32)

    GT = _build_g_matrices()
    Wh = 0.5 * W  # [768, 432]
    perm = np.r_[0:128, 144:272, 288:416, 128:144, 272:288, 416:432]
    WT = np.ascontiguousarray(Wh.T[perm], dtype=np.float32)  # [432, 768]
    b2v = b - np.log(256.0) * W.sum(1)
    B2 = np.ascontiguousarray(b2v.reshape(6, 128).T, dtype=np.float32)  # [128, 6]
    SEL = np.zeros((96, 48), dtype=np.float32)
    for c in range(3):
        for r in range(16):
            SEL[32 * c + r, 16 * c + r] = 1.0
            SEL[32 * c + 16 + r, 16 * c + r] = 1.0

    nc = _build_program()
    in_maps = []
    for m in range(N_CORES):
        xs = x[m * BL : (m + 1) * BL]  # [4, 3, 512, 512]
        xgm = (
            xs.reshape(BL, C, 32, 16, 32, 16)
            .transpose(0, 1, 3, 5, 2, 4)
            .reshape(BL, C, 256, NPAT)
        )
        in_maps.append(
            {
                "xg": np.ascontiguousarray(xgm),
                "gt": GT,
                "wt": WT,
                "b2": B2,
                "sel": SEL,
            }
        )

    import os

    res = run_bass_kernel_spmd(
        nc,
        in_maps,
        list(range(N_CORES)),
        trace=bool(int(os.environ.get("KERNEL_TRACE", "0"))),
    )
    global LAST_RESULT
    LAST_RESULT = res
    outs = [res.results[m]["out"].reshape(BL, E, 32, 32) for m in range(N_CORES)]
    return np.concatenate(outs, 0)


LAST_RESULT = None
